# revision 20
# baseline (speedup 1.0000x reference)
"""Trainium2 Bass kernel for nn_CorrAttentionBias.

Computes out = where(row/col masked, NEG, attn + neigh_band_bias + sink_bias)
for attn_scores [2, 16, 2048, 2048] f32, sharded over (batch, head) across
8 NeuronCores (4 heads of one batch per core).

Device-side math per row-block of 128 rows (i0 = 128*r):
  bias[p, j] = (csink_bcast[p, j] * c_sink[i0+p]) * BETA        (sink outer product)
  bias[p, i0+p-1] += sub[i0+p]; bias[p, i0+p+1] += sup[i0+p]    (neighbor band)
  M[p, j]    = min(maskval[j], maskval[i0+p])                   (NEG if masked else +BIG)
  out[p, j]  = min(attn[p, j] + bias[p, j], M[p, j])            (exact NEG via min)

The min-trick is exact because attn+bias is within a few units of 0 while
NEG = -1e5. All small per-row vectors (band values, mask values) are derived
on host from the tiny [B, L] inputs; all heavy lifting is on device.
"""

import sys

sys.path.insert(0, "/opt/trn_rl_repo")

from contextlib import ExitStack

import numpy as np

import concourse.bass as bass
import concourse.tile as tile
from concourse import bacc, mybir
from concourse.bass_utils import run_bass_kernel_spmd

ALPHA = np.float32(0.5)
BETA = np.float32(0.1)
NEG = np.float32(-100000.0)
BIG = np.float32(3.0e38)

B, H, L = 2, 16, 2048
N_CORES = 8
H_PER = (B * H) // N_CORES  # 4 heads per core
P = 128  # partitions per row-block
N_RB = L // P  # 16 row-blocks

FP = mybir.dt.float32


def _build_program(trace_sim: bool = False) -> bacc.Bacc:
    nc = bacc.Bacc(
        "TRN2",
        target_bir_lowering=False,
        debug=False,
        num_devices=N_CORES,
    )

    attn_d = nc.dram_tensor("attn", [H_PER, L, L], FP, kind="ExternalInput").ap()
    # vecs[:, 0] = c_sink, [:, 1] = maskval, [:, 2] = sub band, [:, 3] = sup band
    vecs_d = nc.dram_tensor("vecs", [L, 4], FP, kind="ExternalInput").ap()
    # rowconsts[0] = c_sink, rowconsts[1] = maskval (broadcast on-chip)
    rowconsts_d = nc.dram_tensor("rowconsts", [2, L], FP, kind="ExternalInput").ap()
    out_d = nc.dram_tensor("out", [H_PER, L, L], FP, kind="ExternalOutput").ap()

    # rows-major views so the partition dim is the row dim
    attn_r = attn_d.rearrange("h r c -> r h c")
    out_r = out_d.rearrange("h r c -> r h c")

    with tile.TileContext(nc, trace_sim=trace_sim) as tc, ExitStack() as ctx:
        const_pool = ctx.enter_context(tc.tile_pool(name="const", bufs=1))
        prep_pool = ctx.enter_context(tc.tile_pool(name="prep", bufs=2))
        band_pool = ctx.enter_context(tc.tile_pool(name="band", bufs=2))
        a_pool = ctx.enter_context(tc.tile_pool(name="a", bufs=8))

        # tiny const loads first on the sync HWDGE FIFO (16 KB, negligible
        # head-of-line cost); on-chip broadcast keeps 2 MiB off HBM
        cs_row = const_pool.tile([1, L], FP, tag="cs_row")
        nc.sync.dma_start(out=cs_row[:, :], in_=rowconsts_d[0:1, :])
        mv_row = const_pool.tile([1, L], FP, tag="mv_row")
        nc.sync.dma_start(out=mv_row[:, :], in_=rowconsts_d[1:2, :])
        # all 16 row-blocks' per-row values: vecs_sb[p, 4*r + k] = vecs[128*r + p, k]
        vecs_sb = const_pool.tile([P, 4 * N_RB], FP, tag="vecs")
        nc.sync.dma_start(
            out=vecs_sb[:, :], in_=vecs_d.rearrange("(r p) k -> p r k", p=P)
        )
        csink_bc = const_pool.tile([P, L], FP, tag="csink_bc")
        nc.gpsimd.partition_broadcast(csink_bc[:, :], cs_row[0:1, :])
        maskval_bc = const_pool.tile([P, L], FP, tag="maskval_bc")
        nc.gpsimd.partition_broadcast(maskval_bc[:, :], mv_row[0:1, :])

        for r in range(N_RB):
            i0 = r * P
            csink_col = vecs_sb[:, 4 * r : 4 * r + 1]
            maskrow_col = vecs_sb[:, 4 * r + 1 : 4 * r + 2]
            sub_col = vecs_sb[:, 4 * r + 2 : 4 * r + 3]
            sup_col = vecs_sb[:, 4 * r + 3 : 4 * r + 4]

            # sink bias on ACT, bitwise-matching reference: round(si*sj) then *BETA
            bias_t = prep_pool.tile([P, L], FP, tag="bias")
            nc.scalar.activation(
                out=bias_t[:, :],
                in_=csink_bc[:, :],
                func=mybir.ActivationFunctionType.Copy,
                scale=csink_col,
            )
            nc.scalar.activation(
                out=bias_t[:, :],
                in_=bias_t[:, :],
                func=mybir.ActivationFunctionType.Copy,
                scale=float(BETA),
            )
            # combined row/col mask values
            m_t = prep_pool.tile([P, L], FP, tag="m")
            nc.vector.tensor_scalar(
                out=m_t[:, :],
                in0=maskval_bc[:, :],
                scalar1=maskrow_col,
                scalar2=None,
                op0=mybir.AluOpType.min,
            )

            # neighbor band: touches cols [i0-1, i0+128] only
            wstart = max(0, i0 - 1)
            wn = min(i0 + P + 1, L) - wstart
            band1 = band_pool.tile([P, 130], FP, tag="band1")
            nc.gpsimd.affine_select(
                out=band1[:, :wn],
                in_=sub_col.broadcast_to([P, wn]),
                pattern=[[1, wn]],
                compare_op=mybir.AluOpType.is_equal,
                fill=0.0,
                base=wstart - i0 + 1,  # keep where q - p + (wstart - i0 + 1) == 0
                channel_multiplier=-1,
            )
            band2 = band_pool.tile([P, 130], FP, tag="band2")
            nc.gpsimd.affine_select(
                out=band2[:, :wn],
                in_=sup_col.broadcast_to([P, wn]),
                pattern=[[1, wn]],
                compare_op=mybir.AluOpType.is_equal,
                fill=0.0,
                base=wstart - i0 - 1,  # keep where q - p + (wstart - i0 - 1) == 0
                channel_multiplier=-1,
            )
            bias_win = bias_t[:, wstart : wstart + wn]
            nc.vector.tensor_tensor(
                out=bias_win, in0=bias_win, in1=band1[:, :wn], op=mybir.AluOpType.add
            )
            nc.vector.tensor_tensor(
                out=bias_win, in0=bias_win, in1=band2[:, :wn], op=mybir.AluOpType.add
            )

            # 4 heads per row-block as two independent 2-head (2 MiB) tiles:
            # finer slot recycling → loads run ahead, stores flush early
            HH = H_PER // 2
            for half in range(2):
                h0 = half * HH
                a_t = a_pool.tile([P, HH * L], FP, tag="a")
                nc.sync.dma_start(
                    out=a_t[:, :],
                    in_=attn_r[i0 : i0 + P, h0 : h0 + HH, :],
                )
                for h in range(HH):
                    a_h = a_t[:, h * L : (h + 1) * L]
                    nc.vector.tensor_tensor(
                        out=a_h, in0=a_h, in1=bias_t[:, :], op=mybir.AluOpType.add
                    )
                    nc.vector.tensor_tensor(
                        out=a_h, in0=a_h, in1=m_t[:, :], op=mybir.AluOpType.min
                    )
                nc.scalar.dma_start(
                    out=out_r[i0 : i0 + P, h0 : h0 + HH, :],
                    in_=a_t[:, :],
                )

    nc.compile()
    return nc


def _host_prep(attn_scores, c_local, c_sink, mask):
    """Slice the full inputs into per-core input maps."""
    attn_scores = np.ascontiguousarray(attn_scores, dtype=np.float32)
    c_local = np.asarray(c_local, dtype=np.float32)
    c_sink = np.asarray(c_sink, dtype=np.float32)
    mask = np.asarray(mask, dtype=bool)

    in_maps = []
    for c in range(N_CORES):
        b = c // (N_CORES // B)
        h0 = H_PER * (c % (N_CORES // B))
        sub = np.zeros(L, np.float32)
        sub[1] = c_local[b, 1]
        sub[L - 1] = c_local[b, L - 1]
        sub[2 : L - 1] = c_local[b, 1 : L - 2]
        sup = np.zeros(L, np.float32)
        sup[: L - 1] = c_local[b, 1:]
        sub = ALPHA * sub
        sup = ALPHA * sup
        maskval = np.where(mask[b], NEG, BIG).astype(np.float32)
        vecs = np.stack([c_sink[b], maskval, sub, sup], axis=1).astype(np.float32)
        in_maps.append(
            {
                "attn": np.ascontiguousarray(attn_scores[b, h0 : h0 + H_PER]),
                "vecs": np.ascontiguousarray(vecs),
                "rowconsts": np.ascontiguousarray(
                    np.stack([c_sink[b], maskval], axis=0)
                ),
            }
        )
    return in_maps


_PROGRAM_CACHE = {}


def _get_program():
    if "nc" not in _PROGRAM_CACHE:
        _PROGRAM_CACHE["nc"] = _build_program()
    return _PROGRAM_CACHE["nc"]


def kernel(attn_scores, c_local, c_sink, mask, _trace=False, _trace_kwargs=None):
    nc = _get_program()
    in_maps = _host_prep(attn_scores, c_local, c_sink, mask)
    res = run_bass_kernel_spmd(
        nc,
        in_maps,
        list(range(N_CORES)),
        trace=_trace,
        **(_trace_kwargs or {}),
    )
    out = np.empty((B, H, L, L), dtype=np.float32)
    for c in range(N_CORES):
        b = c // (N_CORES // B)
        h0 = H_PER * (c % (N_CORES // B))
        out[b, h0 : h0 + H_PER] = res.results[c]["out"]
    kernel.last_results = res
    return out
